# revision 1
# baseline (speedup 1.0000x reference)
"""Trainium2 Bass kernel for dual cross-attention + mean-fuse MLP (CAFM).

Problem: B=16, C=256, H*W=N=2048, DIM=256.
  out_1 = cross_attn(stft_seq, cqt_seq, wq1, wq2, wq3)   # [B, N, C]
  out_2 = cross_attn(cqt_seq, stft_seq, wq4, wq5, wq6)
  fused = concat([mean_n(out_1), mean_n(out_2)])         # [B, 512]
  out   = relu(fused @ W1 + b1) @ W2 + b2                # [B, 256]

Key algebra (exact):
  * softmax is invariant to per-row constants, so
      S = (X Wq + bq)(Y Wk + bk)^T * s  ~  (X A + 1 w^T) Y^T
    with A = s Wq Wk^T, w = s Wk bq — the K projection disappears.
  * only mean_n(softmax(S) V) is needed, so instead of full attn @ V:
      y = p^T V + bv,  p[m] = (1/N) sum_n exp(S[n,m]) / rowsum[n]
    via rinv-weighted reduction matmuls per 128-row block.

Sharding: data-parallel over batch, 2 batch elements per core, both
attention directions per core. No collectives.

Implementation notes:
  * scores matmuls in fp8e4 with DoubleRow perf mode (0.5 cyc/row, K=256
    in one pass); operand scaling: A,w x16 on host, kv-side x(1/16) at the
    fp8 cast, so S is exact up to fp8 mantissa rounding.
  * exp on ScalarE over [128,1024] PSUM (2 banks) with fused accum_out
    row-sums; reciprocal on VectorE.
  * column-sum p: chunks 0-2 via PE matmuls (lhsT=rinv) into three [1,512]
    PSUM accumulators; chunk 3 on VectorE in bf16, partition-reduced by a
    final ones-matmul.
  * PSUM budget: 4 (scores) + 3 (pacc) + 1 (shared scratch) = 8 banks.
"""

import numpy as np

import concourse.bass as bass
import concourse.mybir as mybir
import concourse.tile as tile
from concourse.bass_utils import run_bass_kernel_spmd

F32 = mybir.dt.float32
F32R = mybir.dt.float32r
FP8 = mybir.dt.float8e4
BF16 = mybir.dt.bfloat16
DR = mybir.MatmulPerfMode.DoubleRow
AF = mybir.ActivationFunctionType

N = 2048          # sequence length (H*W)
C = 256           # channels
BLOCKS = N // 128  # 16 row blocks


def split_multi_waits(nc):
    """This container's walrus accepts at most 1 sync-wait per instruction
    (2 for EventSemaphore). Tile's tail drain can carry more; move the
    excess onto preceding wait-only NoOps on the same engine."""
    f = nc.m.functions[0]
    n_new = 0
    for bb in f.blocks:
        insts = bb.instructions
        new_list = []
        changed = False
        for inst in insts:
            si = inst.sync_info
            waits = list(si.on_wait) if si and si.on_wait else []
            cap = 2 if isinstance(inst, mybir.InstEventSemaphore) else 1
            if len(waits) > cap:
                for w in waits[:-cap]:
                    nop = mybir.InstNoOp(
                        name=f"I-sw{n_new}-{inst.name}", ins=[], outs=[])
                    n_new += 1
                    nop.engine = inst.engine
                    nop.sync_info = mybir.SyncInfo(on_wait=[w], on_update=[])
                    new_list.append(nop)
                si.on_wait = waits[-cap:]
                inst.sync_info = si
                changed = True
            new_list.append(inst)
        if changed:
            bb.instructions = new_list
    return n_new


def build_nc(reps=1):
    nc = bass.Bass("TRN2", target_bir_lowering=False, debug=False)

    # --- DRAM I/O (per core) ---
    xq_d = nc.dram_tensor("xq", [2, C, N], F32, kind="ExternalInput")  # stft
    xk_d = nc.dram_tensor("xk", [2, C, N], F32, kind="ExternalInput")  # cqt
    a_d = [nc.dram_tensor(f"a{d}", [C, C], F32, kind="ExternalInput")
           for d in range(2)]
    wt_d = [nc.dram_tensor(f"wt{d}", [C], F32, kind="ExternalInput")
            for d in range(2)]
    wv_d = [nc.dram_tensor(f"wv{d}", [C, C], F32, kind="ExternalInput")
            for d in range(2)]
    bv_d = [nc.dram_tensor(f"bv{d}", [C], F32, kind="ExternalInput")
            for d in range(2)]
    w1_d = nc.dram_tensor("w1", [2 * C, C], F32, kind="ExternalInput")
    b1_d = nc.dram_tensor("b1", [C], F32, kind="ExternalInput")
    w2_d = nc.dram_tensor("w2", [C, C], F32, kind="ExternalInput")
    b2_d = nc.dram_tensor("b2", [C], F32, kind="ExternalInput")
    out_d = nc.dram_tensor("out", [C, 2], F32, kind="ExternalOutput")

    with tile.TileContext(nc) as tc, nc.allow_low_precision(reason="f32r/fp8"):
        with (
            tc.tile_pool(name="const", bufs=1) as const,
            tc.tile_pool(name="seq", bufs=1) as seqp,
            tc.tile_pool(name="tt", bufs=2) as ttp,
            tc.tile_pool(name="vv", bufs=2) as vvp,
            tc.tile_pool(name="ee", bufs=4) as eep,
            tc.tile_pool(name="small", bufs=3) as smallp,
            tc.tile_pool(name="pdve", bufs=2) as pdvep,
            tc.tile_pool(name="ps", bufs=2, space="PSUM") as psp,
            tc.tile_pool(name="pacc", bufs=1, space="PSUM") as paccp,
            tc.tile_pool(name="tv", bufs=2, space="PSUM") as tvp,
        ):
            # --- DMA order: the d=0 weights land first (tiny, needed in
            # the first ~10us), then the big sequence loads; the DMA engines
            # serialize everything, so queue order = arrival order.
            xq_sbs = [seqp.tile([128, 2, N], F32R, tag=f"xq{b}",
                                name=f"xq_sb{b}") for b in range(2)]
            xk_sbs = [seqp.tile([128, 2, N], F32R, tag=f"xk{b}",
                                name=f"xk_sb{b}") for b in range(2)]
            one_sb = const.tile([128, 1], F32)
            nc.vector.memset(one_sb, 1.0)
            one_bf = const.tile([128, 1], BF16)
            nc.vector.memset(one_bf, 1.0)

            a_sb, wt_sb, wv_sb, bv_sb = [], [], [], []
            for d in range(2):
                a = const.tile([128, 2, C], F32R, tag=f"a{d}")
                nc.sync.dma_start(
                    out=a,
                    in_=a_d[d].ap().rearrange("(k p) c -> p k c", p=128).bitcast(F32R))
                a_sb.append(a)
                wt = const.tile([128, 2], F32, tag=f"wt{d}")
                nc.sync.dma_start(
                    out=wt, in_=wt_d[d].ap().rearrange("(t p) -> p t", p=128))
                wt_sb.append(wt)
                wv = const.tile([128, 2, C], F32R, tag=f"wv{d}")
                nc.scalar.dma_start(
                    out=wv,
                    in_=wv_d[d].ap().rearrange("(k p) c -> p k c", p=128).bitcast(F32R))
                wv_sb.append(wv)
                bv = const.tile([1, C], F32, tag=f"bv{d}")
                nc.scalar.dma_start(
                    out=bv, in_=bv_d[d].ap().rearrange("(o c) -> o c", o=1))
                bv_sb.append(bv)
                if d == 0:
                    # first 512 columns of xq0 land first so the first T~
                    # tile starts ~4us in; xk0 (cast critical path) next
                    nc.sync.dma_start(
                        out=xq_sbs[0][:, :, 0:512],
                        in_=xq_d.ap()[0].rearrange(
                            "(k p) n -> p k n", p=128)[:, :, 0:512].bitcast(F32R))
                    nc.scalar.dma_start(
                        out=xk_sbs[0][:, :, 0:1024],
                        in_=xk_d.ap()[0].rearrange(
                            "(k p) n -> p k n", p=128)[:, :, 0:1024].bitcast(F32R))
                    nc.sync.dma_start(
                        out=xq_sbs[0][:, :, 512:2048],
                        in_=xq_d.ap()[0].rearrange(
                            "(k p) n -> p k n", p=128)[:, :, 512:2048].bitcast(F32R))
                    nc.scalar.dma_start(
                        out=xk_sbs[0][:, :, 1024:2048],
                        in_=xk_d.ap()[0].rearrange(
                            "(k p) n -> p k n", p=128)[:, :, 1024:2048].bitcast(F32R))
                    nc.sync.dma_start(
                        out=xq_sbs[1],
                        in_=xq_d.ap()[1].rearrange(
                            "(k p) n -> p k n", p=128).bitcast(F32R))
                    nc.scalar.dma_start(
                        out=xk_sbs[1],
                        in_=xk_d.ap()[1].rearrange(
                            "(k p) n -> p k n", p=128).bitcast(F32R))

            w1_sb = const.tile([128, 4, C], F32)
            nc.sync.dma_start(
                out=w1_sb, in_=w1_d.ap().rearrange("(k p) c -> p k c", p=128))
            b1_sb = const.tile([128, 2], F32)
            nc.sync.dma_start(
                out=b1_sb, in_=b1_d.ap().rearrange("(t p) -> p t", p=128))
            w2_sb = const.tile([128, 2, C], F32)
            nc.scalar.dma_start(
                out=w2_sb, in_=w2_d.ap().rearrange("(k p) c -> p k c", p=128))
            b2_sb = const.tile([128, 2], F32)
            nc.scalar.dma_start(
                out=b2_sb, in_=b2_d.ap().rearrange("(t p) -> p t", p=128))

            # fp8 copies of the sequences (kv-side scores operand), x1/16.
            # b=0 on DVE (startup critical path), b=1 on GpSimd, late.
            xq8s = [seqp.tile([128, 2, N], FP8, tag=f"xq8{b}",
                              name=f"xq8_{b}") for b in range(2)]
            xk8s = [seqp.tile([128, 2, N], FP8, tag=f"xk8{b}",
                              name=f"xk8_{b}") for b in range(2)]
            nc.gpsimd.tensor_scalar_mul(
                xq8s[0], xq_sbs[0].bitcast(F32), 1.0 / 16.0)

            ft_sb = const.tile([128, 8], F32)  # fused^T columns (k-chunk, b)

            tt_tiles = {}

            def emit_tt(b, d, rep, half=None):
                q = xq_sbs[b] if d == 0 else xk_sbs[b]
                aa, ww = a_sb[d], wt_sb[d]
                key = (b, d, rep)
                if key in tt_tiles:
                    t = tt_tiles[key]
                else:
                    t = ttp.tile([128, 2, N], FP8, tag="tt",
                                 name=f"tt{b}{d}_{rep}")
                    tt_tiles[key] = t
                for ct in range(2):
                    if half is not None and ct != half:
                        continue
                    for j4 in range(4):
                        ps = tvp.tile([128, 512], F32, tag="tv",
                                      name=f"ttps{b}{d}{ct}{j4}_{rep}")
                        lo = 512 * j4
                        nc.tensor.matmul(
                            ps, aa[:, 0, ct * 128:(ct + 1) * 128],
                            q[:, 0, lo:lo + 512], start=True, stop=False)
                        nc.tensor.matmul(
                            ps, aa[:, 1, ct * 128:(ct + 1) * 128],
                            q[:, 1, lo:lo + 512], start=False, stop=True)
                        nc.vector.tensor_scalar_add(
                            t[:, ct, lo:lo + 512], ps, ww[:, ct:ct + 1])
                return t

            for _rep in range(reps):
              for b in range(2):
                for d in range(2):
                    if _rep == 0 and b == 1 and d == 0:
                        nc.gpsimd.tensor_scalar_mul(
                            xk8s[1], xk_sbs[1].bitcast(F32), 1.0 / 16.0)
                        nc.gpsimd.tensor_scalar_mul(
                            xq8s[1], xq_sbs[1].bitcast(F32), 1.0 / 16.0)
                    q_seq = xq_sbs[b] if d == 0 else xk_sbs[b]
                    k_seq8 = xk8s[b] if d == 0 else xq8s[b]  # fp8 kv (scores)
                    k_seq = xk_sbs[b] if d == 0 else xq_sbs[b]  # f32r kv (V)
                    a, wt, wv, bv = a_sb[d], wt_sb[d], wv_sb[d], bv_sb[d]

                    # T~^T = (X A + 1 w^T)^T : [c_out 2x128, n], fp8 out.
                    # For iterations after the first, the T~ tiles were
                    # already emitted inside the previous block loop.
                    if (b, d, _rep) in tt_tiles:
                        tt = tt_tiles.pop((b, d, _rep))
                    elif _rep == 0 and b == 0 and d == 0:
                        tt = emit_tt(0, 0, 0)
                        tt_tiles.pop((0, 0, 0))
                        nc.vector.tensor_scalar_mul(
                            xk8s[0][:, :, 0:1024],
                            xk_sbs[0][:, :, 0:1024].bitcast(F32), 1.0 / 16.0)
                        nc.vector.tensor_scalar_mul(
                            xk8s[0][:, :, 1024:2048],
                            xk_sbs[0][:, :, 1024:2048].bitcast(F32), 1.0 / 16.0)
                    else:
                        tt = emit_tt(b, d, _rep)
                        tt_tiles.pop((b, d, _rep))

                    if (b, d, _rep) in tt_tiles:
                        tt = tt_tiles.pop((b, d, _rep))
                    elif _rep == 0 and b == 0 and d == 0:
                        tt = emit_tt(0, 0, 0)
                        tt_tiles.pop((0, 0, 0))
                        nc.vector.tensor_scalar_mul(
                            xk8s[0][:, :, 0:1024],
                            xk_sbs[0][:, :, 0:1024].bitcast(F32), 1.0 / 16.0)
                        nc.vector.tensor_scalar_mul(
                            xk8s[0][:, :, 1024:2048],
                            xk_sbs[0][:, :, 1024:2048].bitcast(F32), 1.0 / 16.0)
                    else:
                        tt = emit_tt(b, d, _rep)
                        tt_tiles.pop((b, d, _rep))
                    # V = Y Wv : [n-block part, c], row-major f32r.
                    # Emitted interleaved into the score-block loop (two
                    # V tiles per early block) so the PSUM-evac copies don't
                    # monopolize DVE's FIFO ahead of the row-sum reduces.
                    v = vvp.tile([128, BLOCKS, C], F32R)

                    def emit_v(mb):
                        ps = tvp.tile([128, 512], F32, tag="tv",
                                      name=f"vps{b}{d}{mb}_{_rep}")
                        nc.tensor.matmul(
                            ps[:, :C], k_seq[:, 0, mb * 128:(mb + 1) * 128],
                            wv[:, 0, :], start=True, stop=False)
                        nc.tensor.matmul(
                            ps[:, :C], k_seq[:, 1, mb * 128:(mb + 1) * 128],
                            wv[:, 1, :], start=False, stop=True)
                        nc.vector.tensor_copy(v[:, mb, :], ps[:, :C])

                    # scores (fp8 DoubleRow) -> exp(+rowsum) -> column sums
                    pacc = [paccp.tile([1, 512], F32, tag=f"pacc{j}",
                                       name=f"pacc{j}_{b}{d}_{_rep}")
                            for j in range(2)]
                    p3 = pdvep.tile([128, 1024], BF16, tag="p3")
                    pending = []
                    for nb in range(BLOCKS):
                        e = eep.tile([128, N], F32R)
                        racc = smallp.tile([128, 2], F32, tag="racc")
                        for j2 in range(2):
                            ps = psp.tile([128, 1024], F32)
                            for jj in range(2):
                                lo = 1024 * j2 + 512 * jj
                                nc.tensor.matmul(
                                    ps[:, 512 * jj:512 * (jj + 1)],
                                    tt[:, :, nb * 128:(nb + 1) * 128],
                                    k_seq8[:, :, lo:lo + 512],
                                    start=True, stop=True, perf_mode=DR)
                            nc.scalar.activation(
                                e[:, 1024 * j2:1024 * (j2 + 1)], ps, AF.Exp,
                                accum_out=racc[:, j2:j2 + 1])
                        rsum = smallp.tile([128, 1], F32, tag="rsum")
                        nc.vector.tensor_reduce(
                            rsum, racc, axis=mybir.AxisListType.X,
                            op=mybir.AluOpType.add)
                        rinv = smallp.tile([128, 1], F32R, tag="rinv")
                        nc.vector.reciprocal(rinv, rsum)
                        rinvf = smallp.tile([128, 1], F32, tag="rinvf")
                        nc.vector.reciprocal(rinvf, rsum)
                        pending.append((e, rinv, rinvf, nb == 0))
                        if nb < 8:
                            emit_v(2 * nb)
                            emit_v(2 * nb + 1)
                        if nb >= 1:
                            pe_, prinv, prinvf, pstart = pending.pop(0)
                            for j in range(2):
                                nc.tensor.matmul(
                                    pacc[j], prinv,
                                    pe_[:, j * 512:(j + 1) * 512],
                                    start=pstart, stop=False,
                                    skip_group_check=True)
                            if pstart:
                                nc.vector.tensor_scalar_mul(
                                    p3, pe_[:, 1024:2048], prinvf)
                            else:
                                cst = pdvep.tile([128, 1024], BF16, tag="cst")
                                nc.vector.tensor_scalar_mul(
                                    cst, pe_[:, 1024:2048], prinvf)
                                nc.vector.tensor_add(p3, p3, cst)
                    pe_, prinv, prinvf, pstart = pending.pop(0)
                    for j in range(2):
                        nc.tensor.matmul(
                            pacc[j], prinv, pe_[:, j * 512:(j + 1) * 512],
                            start=pstart, stop=True, skip_group_check=True)
                    cst = pdvep.tile([128, 1024], BF16, tag="cst")
                    nc.vector.tensor_scalar_mul(cst, pe_[:, 1024:2048], prinvf)
                    nc.vector.tensor_add(p3, p3, cst)

                    # p -> sbuf row [1, 2048]
                    p_sb = smallp.tile([1, N], F32, tag="p")
                    for j in range(2):
                        nc.vector.tensor_copy(
                            p_sb[0:1, j * 512:(j + 1) * 512], pacc[j])
                    for j in range(2):
                        p3ps = paccp.tile([128, 512], F32, tag=f"pacc{j}",
                                          name=f"p3ps{j}_{b}{d}_{_rep}")
                        nc.tensor.matmul(
                            p3ps[0:1, :], one_bf,
                            p3[:, j * 512:(j + 1) * 512], start=True,
                            stop=True, skip_group_check=True)
                        nc.vector.tensor_copy(
                            p_sb[0:1, 1024 + j * 512:1024 + (j + 1) * 512],
                            p3ps[0:1, :])

                    # transpose p via k=1 matmuls, then y = p^T V (+ bv)
                    ptp = paccp.tile([128, 512], F32, tag="pacc0",
                                     name=f"ptp{b}{d}_{_rep}")
                    for j in range(BLOCKS):
                        nc.tensor.matmul(
                            ptp[:, j:j + 1], p_sb[0:1, j * 128:(j + 1) * 128],
                            one_sb[0:1, :], start=(j == 0),
                            stop=(j == BLOCKS - 1), skip_group_check=True)
                    pt_sb = smallp.tile([128, 16], F32R, tag="pt")
                    nc.vector.tensor_copy(pt_sb, ptp[:, :16])
                    yps = paccp.tile([128, 512], F32, tag="pacc1",
                                     name=f"yps{b}{d}_{_rep}")
                    for j in range(BLOCKS):
                        nc.tensor.matmul(
                            yps[0:1, :C], pt_sb[:, j:j + 1], v[:, j, :],
                            start=(j == 0), stop=(j == BLOCKS - 1),
                            skip_group_check=True)
                    y_sb = smallp.tile([1, C], F32, tag="y")
                    nc.vector.tensor_add(y_sb, yps[0:1, :C], bv)

                    # fused^T columns via k=1 transpose matmuls
                    for h in range(2):
                        fcol = paccp.tile([128, 512], F32, tag=f"pacc{h}",
                                          name=f"fcol{b}{d}{h}_{_rep}")
                        nc.tensor.matmul(
                            fcol[:, 0:1], y_sb[0:1, h * 128:(h + 1) * 128],
                            one_sb[0:1, :], start=True, stop=True,
                            skip_group_check=True)
                        k = 2 * d + h
                        nc.vector.tensor_copy(
                            ft_sb[:, 2 * k + b:2 * k + b + 1], fcol[:, 0:1])

            # --- final MLP on the two local batch rows ---
            h_sb = smallp.tile([128, 2, 2], F32, tag="h")
            for t in range(2):
                hps = paccp.tile([128, 512], F32, tag="pacc0", name=f"hps{t}")
                for k in range(4):
                    nc.tensor.matmul(
                        hps[:, 0:2], w1_sb[:, k, t * 128:(t + 1) * 128],
                        ft_sb[:, 2 * k:2 * k + 2],
                        start=(k == 0), stop=(k == 3), skip_group_check=True)
                nc.scalar.activation(
                    h_sb[:, t, :], hps[:, 0:2], AF.Relu,
                    bias=b1_sb[:, t:t + 1], scale=1.0)
            o_sb = smallp.tile([128, 2, 2], F32, tag="o")
            for t in range(2):
                ops = paccp.tile([128, 512], F32, tag="pacc1", name=f"ops{t}")
                for k in range(2):
                    nc.tensor.matmul(
                        ops[:, 0:2], w2_sb[:, k, t * 128:(t + 1) * 128],
                        h_sb[:, k, :],
                        start=(k == 0), stop=(k == 1), skip_group_check=True)
                nc.scalar.activation(
                    o_sb[:, t, :], ops[:, 0:2], AF.Identity,
                    bias=b2_sb[:, t:t + 1], scale=1.0)
            nc.sync.dma_start(
                out=out_d.ap().rearrange("(t p) b -> p t b", p=128), in_=o_sb)

    split_multi_waits(nc)
    return nc


_NC = None


def _get_nc():
    global _NC
    if _NC is None:
        _NC = build_nc()
    return _NC


def prep_inputs(stft_feat, cqt_feat, wq1_w, wq1_b, wq2_w, wq2_b, wq3_w, wq3_b,
                wq4_w, wq4_b, wq5_w, wq5_b, wq6_w, wq6_b,
                out1_w, out1_b, out2_w, out2_b):
    B = stft_feat.shape[0]
    s = 1.0 / np.sqrt(np.float32(C))
    f32 = np.float32
    sigma = np.float32(16.0)  # fp8 range balancing; kv-side scaled by 1/16
    A1 = (wq1_w @ wq2_w.T * s * sigma).astype(f32)
    wt1 = (wq2_w @ wq1_b * s * sigma).astype(f32)
    A2 = (wq4_w @ wq5_w.T * s * sigma).astype(f32)
    wt2 = (wq5_w @ wq4_b * s * sigma).astype(f32)
    WV1 = (wq3_w / f32(N)).astype(f32)
    WV2 = (wq6_w / f32(N)).astype(f32)
    common = dict(
        a0=np.ascontiguousarray(A1), a1=np.ascontiguousarray(A2),
        wt0=np.ascontiguousarray(wt1), wt1=np.ascontiguousarray(wt2),
        wv0=np.ascontiguousarray(WV1), wv1=np.ascontiguousarray(WV2),
        bv0=np.ascontiguousarray(wq3_b.astype(f32)),
        bv1=np.ascontiguousarray(wq6_b.astype(f32)),
        w1=np.ascontiguousarray(out1_w.astype(f32)),
        b1=np.ascontiguousarray(out1_b.astype(f32)),
        w2=np.ascontiguousarray(out2_w.astype(f32)),
        b2=np.ascontiguousarray(out2_b.astype(f32)),
    )
    stft = np.ascontiguousarray(stft_feat.reshape(B, C, N).astype(f32))
    cqt = np.ascontiguousarray(cqt_feat.reshape(B, C, N).astype(f32))
    in_maps = []
    for i in range(8):
        m = dict(common)
        m["xq"] = np.ascontiguousarray(stft[2 * i:2 * i + 2])
        m["xk"] = np.ascontiguousarray(cqt[2 * i:2 * i + 2])
        in_maps.append(m)
    return in_maps


def kernel(**inputs):
    inputs = {k: np.asarray(v) for k, v in inputs.items()}
    B = inputs["stft_feat"].shape[0]
    nc = _get_nc()
    in_maps = prep_inputs(**inputs)
    res = run_bass_kernel_spmd(nc, in_maps, list(range(8)))
    out = np.empty((B, C), np.float32)
    for i in range(8):
        o = res.results[i]["out"]  # [C, 2]
        out[2 * i] = o[:, 0]
        out[2 * i + 1] = o[:, 1]
    return out

